# revision 36
# baseline (speedup 1.0000x reference)
"""Trainium2 Bass kernel for CapsNet dynamic routing (nn_CapsRoutingLayer).

Reference computation:
    x_hat[b,i,o,d] = sum_k W[i,o,d,k] * x[b,i,k]
    b_logits = 0
    for it in 0..2:
        c = softmax_o(b_logits); s[b,o,d] = sum_i c[b,i,o] x_hat[b,i,o,d]
        v = squash(s)   # global Frobenius norm over the whole s tensor
        if it < 2: b_logits += sum_d x_hat[b,i,o,d] v[b,o,d]
    return v  # (128, 32, 32)

Sharding: input capsules i (1152) split across 8 cores (144 each); the tiny
[128,1024] per-iteration s is AllReduced. Per-core strategy:

  * All matmuls in bf16 (PE runs fp32 at 1/4 rate). W shard (9.4MB bf16)
    stays resident in SBUF, loaded in 4 chunks so pass-0 starts early.
  * Pass 0 (uniform c): s0 = (1/32) sum_i x_hat via K=128 matmuls.
  * Passes 1,2 regenerate x_hat per supergroup of 8 capsules into PSUM
    (2-capsule tiles, double-buffered), the Scalar engine copies it to SBUF
    as bf16, and all elementwise work runs on DVE in its 2x/4x 16-bit modes.
    Per-capsule s contributions are pair-summed in a small bf16 add tree on
    DVE; GpSimd (0.42 add efficiency) only accumulates one fp32 add per 16
    capsules.
  * squash's global norm is PSUM-free: tensor_tensor_reduce + GpSimd
    partition reduce/broadcast. Routing logits are linear in v, so pass 2
    uses m2 = v0 + v1 as its agreement multiplier (b2 = <x_hat, v0+v1>).

Software pipeline (emission order == per-engine execution order): supergroup
S's PE regen + ACT copies are emitted one round ahead of its softmax and s
accumulation, so no engine stalls on another's tail.
"""

import numpy as np
import ml_dtypes

from concourse import bacc, bass_isa, bass_utils, mybir, tile

N_CORES = 8
B = 128          # batch
NI = 1152        # input capsules
K = 32           # dim_input
NO = 32          # output capsules
D = 32           # dim_output
IC = NI // N_CORES   # input capsules per core = 144
NJ = IC // 4         # i-groups of 4 per core = 36
OD = NO * D          # 1024
SGC = 8              # capsules per supergroup
NSG = IC // SGC      # supergroups per pass = 18
NWC = 4              # W is DMA'd in NWC chunks of NJ//NWC j-groups
JC = NJ // NWC       # 9

F32 = mybir.dt.float32
BF16 = mybir.dt.bfloat16
ADD = mybir.AluOpType.add
MULT = mybir.AluOpType.mult
AXX = mybir.AxisListType.X
AXC = mybir.AxisListType.C
EXP = mybir.ActivationFunctionType.Exp

# Timing-ablation only: replace the cross-core AllReduce with a plain DMA
# (results become wrong; used to measure the collective's cost).
SKIP_COLLECTIVE = False
# Debug toggles for HW bisection.
USE_TTR = False     # tensor_tensor_reduce in squash
USE_STT = True


def _kernel_body(nc, tc, xs, ws, vout, repeats=1):
    with tc.tile_pool(name="persist", bufs=1) as per, \
         tc.tile_pool(name="xhp", bufs=3) as xhp, \
         tc.tile_pool(name="smallp", bufs=2) as smallp, \
         tc.tile_pool(name="pgp", bufs=2, space="PSUM") as pgp, \
         tc.tile_pool(name="dram", bufs=1, space="DRAM") as dram:

        W_t = [per.tile([128, JC, OD], BF16, name=f"W_t{c}") for c in range(NWC)]
        x_t = per.tile([128, NJ, 128], BF16)
        nc.sync.dma_start(x_t[:], xs[:])       # small; every matmul needs it
        for c in range(NWC):
            nc.sync.dma_start(W_t[c][:], ws[:, JC * c:JC * (c + 1), :])

        xv = per.tile([B, SGC * OD], BF16)     # agreement products (DVE-only)
        sx = per.tile([B, SGC * OD], BF16)     # c-weighted x_hat (DVE-only)
        s_acc = per.tile([B, OD], F32)         # local s accumulator
        s_loc = per.tile([B, OD], F32)         # pass-0 staging
        s_full = per.tile([B, OD], F32)        # post-AllReduce s
        v0 = per.tile([B, OD], F32)            # squash(s0)
        v_out = per.tile([B, OD], F32)         # final output
        m_bf = per.tile([B, OD], BF16)         # agreement multiplier (bf16)
        sq = per.tile([B, OD], F32)            # squash scratch
        col = per.tile([B, 1], F32)
        Sn1 = per.tile([1, 1], F32)
        Sb = per.tile([128, 1], F32)
        t1 = per.tile([128, 1], F32)
        t2 = per.tile([128, 1], F32)
        t3 = per.tile([128, 1], F32)
        gb = per.tile([128, 1], F32)
        ones128 = per.tile([128, 1], F32)
        ones1 = per.tile([1, 128], F32)
        nc.vector.memset(ones128[:], 1.0)
        nc.vector.memset(ones1[:], 1.0)

        ar_in = dram.tile([B, OD], F32)
        ar_out = dram.tile([B, OD], F32)

        def allreduce(src):
            nc.sync.dma_start(ar_in[:], src[:])
            if SKIP_COLLECTIVE:
                nc.sync.dma_start(ar_out[:], ar_in[:])
            else:
                nc.gpsimd.collective_compute(
                    "AllReduce", ADD,
                    replica_groups=[list(range(N_CORES))],
                    ins=[ar_in.opt()], outs=[ar_out.opt()],
                )
            nc.sync.dma_start(s_full[:], ar_out[:])

        def squash_mult(pass_idx):
            # g = sqrt(S)/(1+S) with S = global sum of squares of s_full,
            # then update the next pass's agreement multiplier / output.
            if USE_TTR:
                nc.vector.tensor_tensor_reduce(
                    out=sq[:], in0=s_full[:], in1=s_full[:], scale=1.0,
                    scalar=0.0, op0=MULT, op1=ADD, accum_out=col[:])
            else:
                nc.vector.tensor_mul(sq[:], s_full[:], s_full[:])
                nc.vector.tensor_reduce(out=col[:], in_=sq[:], axis=AXX,
                                        op=ADD)
            # partition reduce + broadcast via PE with ones (1-row matmuls)
            ps = pgp.tile([B, 2 * OD], F32, name="ps", tag="pg")
            nc.tensor.matmul(ps[0:1, 0:1], ones128[:], col[:],
                             start=True, stop=True)
            nc.vector.tensor_copy(Sn1[:], ps[0:1, 0:1])
            nc.tensor.matmul(ps[:, 512:513], ones1[:], Sn1[:],
                             start=True, stop=True)
            nc.vector.tensor_copy(Sb[:], ps[:, 512:513])
            nc.scalar.sqrt(t1[:], Sb[:])
            nc.vector.tensor_scalar_add(t2[:], Sb[:], 1.0)
            nc.vector.reciprocal(t3[:], t2[:])
            nc.vector.tensor_mul(gb[:], t1[:], t3[:])
            if pass_idx == 0:
                nc.vector.tensor_scalar_mul(v0[:], s_full[:], gb[:])
                nc.vector.tensor_copy(m_bf[:], v0[:])          # m1 = v0
            elif pass_idx == 1:
                if USE_STT:
                    nc.vector.scalar_tensor_tensor(             # m2 = g1*s1+v0
                        out=m_bf[:], in0=s_full[:], scalar=gb[:], in1=v0[:],
                        op0=MULT, op1=ADD)
                else:
                    nc.vector.tensor_scalar_mul(sq[:], s_full[:], gb[:])
                    nc.vector.tensor_add(m_bf[:], sq[:], v0[:])
            else:
                nc.vector.tensor_scalar_mul(v_out[:], s_full[:], gb[:])

        def regen_supergroup(S):
            # PE: x_hat for capsules 8S..8S+7 -> PSUM; ACT: cast-copy to SBUF
            xh = xhp.tile([B, SGC * OD], BF16, name="xh", tag="xh")
            for g4 in range(SGC // 2):
                pg = pgp.tile([B, 2 * OD], F32, name="pg", tag="pg")
                for slot in range(2):
                    i = SGC * S + 2 * g4 + slot
                    j, gg = divmod(i, 4)
                    wt = W_t[j // JC]
                    jj = j % JC
                    for h in range(2):
                        lo = slot * OD + 512 * h
                        nc.tensor.matmul(
                            pg[:, lo:lo + 512],
                            x_t[32 * gg:32 * (gg + 1), j, :],
                            wt[32 * gg:32 * (gg + 1), jj, 512 * h:512 * (h + 1)],
                            start=True, stop=True, tile_position=(32 * gg, 0))
                nc.scalar.copy(xh[:, 2 * OD * g4:2 * OD * (g4 + 1)], pg[:])
            return xh

        def run_pass(r):
            state = {}
            for S in range(NSG + 1):
                if S < NSG:
                    xh = regen_supergroup(S)
                    # agreement multiply: xv = xh * m (broadcast over capsule)
                    nc.vector.tensor_tensor(
                        out=xv[:].rearrange("b (i f) -> b i f", i=SGC),
                        in0=xh[:].rearrange("b (i f) -> b i f", i=SGC),
                        in1=m_bf[:].unsqueeze(1).broadcast_to([B, SGC, OD]),
                        op=MULT)
                if S >= 1:
                    st = state[S - 1]
                    nc.vector.tensor_reduce(
                        out=st["z"][:],
                        in_=st["e"][:].rearrange("b (i o) -> b i o", i=SGC),
                        axis=AXX, op=ADD)
                    nc.vector.reciprocal(st["rz"][:], st["z"][:])
                if S < NSG:
                    # reduce over d (the MIDDLE axis in the [b,i,d,o] layout,
                    # so every fold keeps a packed o innermost and runs in the
                    # DVE 2x mode; TensorReduce has no fast mode at all) as a
                    # log2 fold tree of adds.
                    a16 = smallp.tile([B, SGC * NO], BF16, name="a16", tag="a")
                    v4 = xv[:].rearrange("b (i d o) -> b i d o", i=SGC, d=D)
                    w = D // 2
                    while w > 1:
                        nc.vector.tensor_add(v4[:, :, 0:w, :], v4[:, :, 0:w, :],
                                             v4[:, :, w:2 * w, :])
                        w //= 2
                    nc.vector.tensor_add(
                        a16[:].rearrange("b (i o) -> b i o", i=SGC)
                            .unsqueeze(2),
                        v4[:, :, 0:1, :], v4[:, :, 1:2, :])
                    e16 = smallp.tile([B, SGC * NO], BF16, name="e16", tag="e")
                    nc.scalar.activation(e16[:], a16[:], EXP)
                    z16 = smallp.tile([B, SGC], F32, name="z16", tag="z")
                    rz16 = smallp.tile([B, SGC], F32, name="rz16", tag="rz")
                    state[S] = dict(e=e16, z=z16, rz=rz16, xh=xh)
                if S >= 1:
                    st = state[S - 1]
                    c16 = smallp.tile([B, SGC * NO], BF16, name="c16", tag="c")
                    nc.gpsimd.tensor_tensor(
                        out=c16[:].rearrange("b (i o) -> b i o", i=SGC),
                        in0=st["e"][:].rearrange("b (i o) -> b i o", i=SGC),
                        in1=st["rz"][:].unsqueeze(2).broadcast_to([B, SGC, NO]),
                        op=MULT)
                    # s contribution: sx = xh * c (broadcast over d; d is the
                    # middle axis so the innermost o stays packed -> DVE 2x)
                    nc.vector.tensor_tensor(
                        out=sx[:].rearrange("b (i d o) -> b i d o", i=SGC, d=D),
                        in0=st["xh"][:].rearrange("b (i d o) -> b i d o",
                                                  i=SGC, d=D),
                        in1=c16[:].rearrange("b (i o) -> b i o", i=SGC)
                            .unsqueeze(2).broadcast_to([B, SGC, D, NO]),
                        op=MULT)
                    # capsule-sum tree, split DVE (blocks 0-3) / GpSimd (4-7);
                    # GpSimd then owns the serial fp32 s_acc accumulation, so
                    # DVE never waits on the slower engine.
                    nc.vector.tensor_add(sx[:, 0:OD], sx[:, 0:OD],
                                         sx[:, OD:2 * OD])
                    nc.vector.tensor_add(sx[:, 2 * OD:3 * OD],
                                         sx[:, 2 * OD:3 * OD],
                                         sx[:, 3 * OD:4 * OD])
                    nc.vector.tensor_add(sx[:, 0:OD], sx[:, 0:OD],
                                         sx[:, 2 * OD:3 * OD])
                    nc.gpsimd.tensor_add(sx[:, 4 * OD:5 * OD],
                                         sx[:, 4 * OD:5 * OD],
                                         sx[:, 5 * OD:6 * OD])
                    nc.gpsimd.tensor_add(sx[:, 6 * OD:7 * OD],
                                         sx[:, 6 * OD:7 * OD],
                                         sx[:, 7 * OD:8 * OD])
                    nc.gpsimd.tensor_add(sx[:, 4 * OD:5 * OD],
                                         sx[:, 4 * OD:5 * OD],
                                         sx[:, 6 * OD:7 * OD])
                    if S - 1 == 0:
                        nc.gpsimd.tensor_add(s_acc[:], sx[:, 0:OD],
                                             sx[:, 4 * OD:5 * OD])
                    else:
                        nc.gpsimd.tensor_add(s_acc[:], s_acc[:], sx[:, 0:OD])
                        nc.gpsimd.tensor_add(s_acc[:], s_acc[:],
                                             sx[:, 4 * OD:5 * OD])
            allreduce(s_acc)
            squash_mult(r)

        with nc.allow_low_precision("bf16 routing; tolerance is 2e-2"):
            for _rep in range(repeats):
                # ---- pass 0: s0 = (1/32) sum_i x_hat, K=128 matmuls
                pg0 = pgp.tile([B, 2 * OD], F32, name="pg0", tag="pg")
                for j in range(NJ):
                    wt = W_t[j // JC]
                    jj = j % JC
                    for h in range(2):
                        nc.tensor.matmul(
                            pg0[:, 512 * h:512 * (h + 1)],
                            x_t[:, j, :], wt[:, jj, 512 * h:512 * (h + 1)],
                            start=(j == 0), stop=(j == NJ - 1))
                nc.vector.tensor_scalar_mul(s_loc[:], pg0[:, 0:OD], 1.0 / NO)
                allreduce(s_loc)
                squash_mult(0)
                run_pass(1)
                run_pass(2)

        nc.sync.dma_start(vout[:], v_out[:])


_NC_CACHE = {}


def _build(repeats=1):
    if repeats in _NC_CACHE:
        return _NC_CACHE[repeats]
    nc = bacc.Bacc("TRN2", target_bir_lowering=False, debug=False,
                   num_devices=N_CORES)
    xs = nc.dram_tensor("xs", [128, NJ, 128], BF16, kind="ExternalInput").ap()
    ws = nc.dram_tensor("ws", [128, NJ, OD], BF16, kind="ExternalInput").ap()
    vout = nc.dram_tensor("v", [B, OD], F32, kind="ExternalOutput").ap()
    with tile.TileContext(nc) as tc:
        _kernel_body(nc, tc, xs, ws, vout, repeats=repeats)
    nc.compile()
    _NC_CACHE[repeats] = nc
    return nc


def _shard_inputs(x, W):
    BF = ml_dtypes.bfloat16
    in_maps = []
    for c in range(N_CORES):
        i0 = c * IC
        wc = W[i0:i0 + IC]                          # (144, 32, 32, 32) iodk
        # (d,o)-transposed columns: ws[(g,k), j, (d,o)] = W[i0+4j+g, o, d, k]
        wsn = np.ascontiguousarray(
            wc.reshape(NJ, 4, NO, D, K).transpose(1, 4, 0, 3, 2)
              .reshape(128, NJ, OD)).astype(BF)
        xc = x[:, i0:i0 + IC, :]                    # (128, 144, 32) bik
        xt = np.ascontiguousarray(
            xc.reshape(B, NJ, 4, K).transpose(2, 3, 1, 0)
              .reshape(128, NJ, 128)).astype(BF)
        in_maps.append({"xs": xt, "ws": wsn})
    return in_maps


def kernel(x, W, _trace=False):
    x = np.asarray(x, dtype=np.float32)
    W = np.asarray(W, dtype=np.float32)
    nc = _build()
    in_maps = _shard_inputs(x, W)
    res = bass_utils.run_bass_kernel_spmd(
        nc, in_maps, core_ids=list(range(N_CORES)), trace=_trace)
    # kernel works in (d,o)-transposed layout; untranspose on the host
    out = np.ascontiguousarray(
        res.results[0]["v"].reshape(B, D, NO).transpose(0, 2, 1)
    ).astype(np.float32, copy=False)
    if _trace:
        kernel.last_exec_time_ns = res.exec_time_ns
        kernel.last_results = res
    return out
